# revision 11
# baseline (speedup 1.0000x reference)
"""Trainium2 Bass kernel for nn_AttentionLayer_19782619365684.

Computes, for h[32,1024], v[32,2048,512], W1[512,1024], b1[512], W2[512,512],
b2[512], w3[512]:
    hp = h @ W1.T + b1                      # [B, P]
    vp = einsum('bfp,qp->bfq', v, W2) + b2  # [B, F, P]
    e  = einsum('bfp,p->bf', tanh(hp[:,None,:] + vp), w3)
    a  = softmax(e.T.reshape(-1).reshape(B, F), axis=1)

Strategy (8 NeuronCores, data parallel over frames, zero communication):
  - Shard F=2048 frames -> 256 per core. Scrambled output row i needs
    e[b, f] for f in [64i, 64(i+1)) over all b, so core c (frames
    [256c, 256c+256)) owns exactly output rows [4c, 4c+4).
  - Rows within a core are ordered f-major (r = f_local*32 + b). In that
    order the scramble flatten is the identity: e_flat[r] is already the
    scrambled row-major stream, so group g of 512 rows lands at output
    row g//4, columns [512*(g%4), 512*(g%4)+512).
  - v is pre-transposed on the host to [P, rows] so the contraction dim
    sits on SBUF partitions; matmuls run as f32r (full PE rate at N=512).
  - hp (+b1+b2) is computed on-device transposed ([q, b]), replicated
    along free into a [128, 4, 512] bias tile; DVE adds it to the matmul
    PSUM, ACT applies tanh, and the w3 dot is a skinny M=4 matmul whose
    PSUM rows are broadcast copies of e, so row g//4 can be exp'ed
    straight into the scrambled buffer with an accumulated row-sum.
"""

import os
import sys

import numpy as np

for _p in ("/opt/trn_rl_repo", "/root/.axon_site/_ro/trn_rl_repo"):
    if os.path.isdir(_p) and _p not in sys.path:
        sys.path.insert(0, _p)

import concourse.bacc as bacc
import concourse.bass as bass
import concourse.tile as tile
from concourse import mybir
from concourse.bass_utils import run_bass_kernel_spmd

B = 32          # batch
F = 2048        # num frames (global)
H = 1024        # h hidden dim
P = 512         # v feature dim / W2 dim
NCORES = 8
FL = F // NCORES            # frames per core = 256
R = B * FL                  # rows per core = 8192
GR = 512                    # rows per compute group
NG = R // GR                # compute groups = 16
SG_ROWS = 2048              # rows per DMA super-group
GPSG = SG_ROWS // GR        # compute groups per super-group = 4
QC = P // 128               # q chunks = 4
PC = P // 128               # p chunks = 4
KC = H // 128               # k chunks for the hp matmul = 8

F32 = mybir.dt.float32
F32R = mybir.dt.float32r
BF16 = mybir.dt.bfloat16
AF = mybir.ActivationFunctionType

TRACE = False           # set True (from test.py) to capture an NTFF profile
LAST_RESULTS = None     # BassKernelResults of the most recent run


def build_nc():
    nc = bacc.Bacc("TRN2", target_bir_lowering=False)

    vt = nc.declare_dram_parameter("vt", [P, R], F32R, isOutput=False)[:]
    w2t = nc.declare_dram_parameter("w2t", [P, P], F32R, isOutput=False)[:]
    w1t = nc.declare_dram_parameter("w1t", [H, P], F32, isOutput=False)[:]
    ht = nc.declare_dram_parameter("ht", [H, B], F32, isOutput=False)[:]
    b1r = nc.declare_dram_parameter("b1r", [128, QC], F32, isOutput=False)[:]
    b2r = nc.declare_dram_parameter("b2r", [128, QC], F32, isOutput=False)[:]
    w3r = nc.declare_dram_parameter("w3r", [128, QC, 4], BF16, isOutput=False)[:]
    out = nc.declare_dram_parameter("out", [4, F], F32, isOutput=True)[:]

    with tile.TileContext(nc) as tc:
        with (
            tc.tile_pool(name="singles", bufs=1) as singles,
            tc.tile_pool(name="vt_pool", bufs=2) as vtp,
            tc.tile_pool(name="z_pool", bufs=2) as zp,
            tc.tile_pool(name="x_pool", bufs=2) as xp,
            tc.tile_pool(name="vp_psum", bufs=4, space="PSUM") as vpp,
            tc.tile_pool(name="e_psum", bufs=2, space="PSUM") as epp,
            tc.tile_pool(name="hp_psum", bufs=1, space="PSUM") as hpp,
        ):
            # ---- one-time loads ----
            w2t_sb = singles.tile([128, PC, P], F32R)
            nc.sync.dma_start(w2t_sb[:], w2t.rearrange("(po pi) q -> pi po q", pi=128))
            w1t_sb = singles.tile([128, KC, P], F32)
            nc.sync.dma_start(w1t_sb[:], w1t.rearrange("(ko ki) q -> ki ko q", ki=128))
            ht_sb = singles.tile([128, KC, B], F32)
            nc.sync.dma_start(ht_sb[:], ht.rearrange("(ko ki) b -> ki ko b", ki=128))
            b1_sb = singles.tile([128, QC], F32)
            nc.sync.dma_start(b1_sb[:], b1r)
            b2_sb = singles.tile([128, QC], F32)
            nc.sync.dma_start(b2_sb[:], b2r)
            w3_sb = singles.tile([128, QC, 4], BF16)
            nc.sync.dma_start(w3_sb[:], w3r)

            b12 = singles.tile([128, QC], F32)
            nc.vector.tensor_add(b12[:], b1_sb[:], b2_sb[:])

            # ---- hpbT[q, b] = (W1 @ h.T)[q, b] + b1[q] + b2[q], replicated
            #      along free to cover 512 rows (b cycles every 32) ----
            brep = singles.tile([128, QC, GR], F32)
            for qc in range(QC):
                ps = hpp.tile([128, B], F32)
                for kc in range(KC):
                    nc.tensor.matmul(
                        ps[:],
                        lhsT=w1t_sb[:, kc, 128 * qc : 128 * (qc + 1)],
                        rhs=ht_sb[:, kc, :],
                        start=(kc == 0),
                        stop=(kc == KC - 1),
                    )
                nc.vector.tensor_scalar_add(
                    brep[:, qc, 0:B], ps[:], scalar1=b12[:, qc : qc + 1]
                )
                w = B
                while w < GR:
                    nc.vector.tensor_copy(brep[:, qc, w : 2 * w], brep[:, qc, 0:w])
                    w *= 2

            # exp(e) in scrambled flat order: with f-major rows, group g is
            # exactly columns [512g, 512g+512) of the row-major scrambled
            # stream, all kept on partition 0 (PSUM 1-partition reads are
            # only legal at partition 0).
            scram = singles.tile([1, R], F32)
            gsum = singles.tile([1, NG], F32)   # per-group partial row sums

            def w3_stage(x_tile, g):
                # e = w3 . x, contracting q on partitions; lhsT is w3
                # replicated to M=4 columns, partition 0 of PSUM holds e.
                e_ps = epp.tile([4, GR], F32)
                for qc in range(QC):
                    nc.tensor.matmul(
                        e_ps[:],
                        lhsT=w3_sb[:, qc, :],
                        rhs=x_tile[:, qc, :],
                        start=(qc == 0),
                        stop=(qc == QC - 1),
                    )
                nc.scalar.activation(
                    scram[0:1, GR * g : GR * (g + 1)],
                    e_ps[0:1, :],
                    AF.Exp,
                    accum_out=gsum[0:1, g : g + 1],
                )

            pend = None
            for sg in range(R // SG_ROWS):
                vt_sb = vtp.tile([128, PC, SG_ROWS], F32R)
                for pc in range(PC):
                    nc.sync.dma_start(
                        vt_sb[:, pc, :],
                        vt[128 * pc : 128 * (pc + 1),
                           SG_ROWS * sg : SG_ROWS * (sg + 1)],
                    )
                for lg in range(GPSG):
                    g = sg * GPSG + lg
                    z = zp.tile([128, QC, GR], F32)
                    x = xp.tile([128, QC, GR], BF16)
                    for qc in range(QC):
                        vp = vpp.tile([128, GR], F32)
                        for pc in range(PC):
                            nc.tensor.matmul(
                                vp[:],
                                lhsT=w2t_sb[:, pc, 128 * qc : 128 * (qc + 1)],
                                rhs=vt_sb[:, pc, GR * lg : GR * (lg + 1)],
                                start=(pc == 0),
                                stop=(pc == PC - 1),
                            )
                        nc.vector.tensor_add(z[:, qc, :], vp[:], brep[:, qc, :])
                    nc.scalar.activation(x[:], z[:], AF.Tanh)
                    if pend is not None:
                        w3_stage(*pend)
                    pend = (x, g)
            w3_stage(*pend)

            # ---- softmax tail: scale by reciprocal row sums, write out ----
            # Each output row i covers groups [4i, 4i+4); split the scaling
            # of the four rows across DVE and ACT so they run in parallel.
            stot = singles.tile([1, 4], F32)
            for i in range(4):
                nc.vector.reduce_sum(
                    stot[0:1, i : i + 1], gsum[0:1, 4 * i : 4 * i + 4],
                    axis=mybir.AxisListType.X,
                )
            rinv = singles.tile([1, 4], F32)
            nc.vector.reciprocal(rinv[:], stot[:])
            for i in range(4):
                sl = scram[0:1, F * i : F * (i + 1)]
                if i < 2:
                    nc.vector.tensor_scalar_mul(sl, sl, scalar1=rinv[0:1, i : i + 1])
                else:
                    nc.scalar.mul(sl, sl, mul=rinv[0:1, i : i + 1])
            nc.sync.dma_start(out.rearrange("r f -> (r f)"), scram[0:1, :])

    nc.compile()
    return nc


def make_in_maps(inputs):
    h = np.asarray(inputs["h"], dtype=np.float32)
    v = np.asarray(inputs["v"], dtype=np.float32)
    W1 = np.asarray(inputs["W1"], dtype=np.float32)
    b1 = np.asarray(inputs["b1"], dtype=np.float32)
    W2 = np.asarray(inputs["W2"], dtype=np.float32)
    b2 = np.asarray(inputs["b2"], dtype=np.float32)
    w3 = np.asarray(inputs["w3"], dtype=np.float32)

    ht = np.ascontiguousarray(h.T)                       # [H, B]
    w1t = np.ascontiguousarray(W1.T)                     # [H, P]
    w2t = np.ascontiguousarray(W2.T)                     # [P, P]
    b1r = np.ascontiguousarray(b1.reshape(QC, 128).T)    # [128, QC]
    b2r = np.ascontiguousarray(b2.reshape(QC, 128).T)    # [128, QC]
    import ml_dtypes

    w3r = np.ascontiguousarray(
        np.broadcast_to(w3.reshape(QC, 128).T[:, :, None], (128, QC, 4))
    ).astype(ml_dtypes.bfloat16)

    in_maps = []
    for c in range(NCORES):
        vs = v[:, c * FL : (c + 1) * FL, :]              # [B, FL, P]
        # [P, FL, B] -> [P, R]: row index r = f_local*B + b (f-major)
        vtc = np.ascontiguousarray(vs.transpose(2, 1, 0).reshape(P, R))
        in_maps.append(
            {"vt": vtc, "w2t": w2t, "w1t": w1t, "ht": ht,
             "b1r": b1r, "b2r": b2r, "w3r": w3r}
        )
    return in_maps


_NC_CACHE = None


def kernel(**inputs) -> np.ndarray:
    global _NC_CACHE, LAST_RESULTS
    if _NC_CACHE is None:
        _NC_CACHE = build_nc()
    nc = _NC_CACHE
    in_maps = make_in_maps(inputs)
    res = run_bass_kernel_spmd(nc, in_maps, core_ids=list(range(NCORES)),
                               trace=TRACE)
    LAST_RESULTS = res
    outs = [np.asarray(res.results[c]["out"]) for c in range(NCORES)]
    return np.concatenate(outs, axis=0).astype(np.float32)  # [B, F]
